# revision 15
# baseline (speedup 1.0000x reference)
"""Coarse-Fine self-attention layer on 8 Trainium2 NeuronCores.

Data-parallel over batch: 16 batches -> 2 per core. Weights replicated.

Math notes (vs the reference):
  - softmax over keys is invariant to per-query constants, so q_proj and
    pos_b drop out; only k_proj (per-key) matters in the energy.
  - BatchNorm (inference form) folds into trans_w / trans_b on the host.
  - The query-normalization division (att / (1e-9 + colsum)) is folded into
    x_v as a per-key scale after transposing the attention matrix.

dtype discipline: f32 tiles feeding matmuls are bitcast to float32r at both
the writer and the matmul operand (PE streams f32r at 4x the plain-fp32
rate); the attention matrix itself runs in bf16 (ATT=bf16) so transposes run
at 1 cyc/row and the x_r matmul takes N=1024 moving tiles.

Engine split per batch: PE matmuls+transposes; DVE k-sub/rowmax/psum copies;
ACT exp(+rowsum accum)/Dinv scale/ReLU; GPSIMD Rinv scale + residual add.
Batches are emitted interleaved (A0 A1 B0 C0 B1 D0 C1 D1) so batch 1's
softmax overlaps batch 0's PE-heavy output phases.
"""

import os
import numpy as np
from contextlib import ExitStack

import ml_dtypes
from concourse import bacc, tile, mybir
from concourse.bass_utils import run_bass_kernel_spmd

dt = mybir.dt
F32 = dt.float32
F32R = dt.float32r
BF16 = dt.bfloat16
AF = mybir.ActivationFunctionType
ALU = mybir.AluOpType

B = 16          # total batches
C = 512         # channels
NQ = 1024       # queries
NK = 1024       # keys
CQ = 128        # C // 4, q/k projection dim
NCORES = 8
BPC = B // NCORES  # batches per core

MM_DT = F32R if os.environ.get("MMDT", "f32r") == "f32r" else F32
ATT_BF16 = os.environ.get("ATT", "bf16") == "bf16"
ATT_DT = BF16 if ATT_BF16 else F32


def _r(ap):
    return ap.bitcast(MM_DT) if MM_DT is not F32 else ap


def build_program():
    nc = bacc.Bacc(
        "TRN2",
        target_bir_lowering=False,
        debug=False,
        enable_asserts=False,
        num_devices=NCORES,
    )

    x_d = nc.dram_tensor("x", [BPC, C, 2048], F32, kind="ExternalInput").ap()
    kp_d = nc.dram_tensor("kp", [BPC, 1, NK], F32, kind="ExternalInput").ap()
    wq_d = nc.dram_tensor("wq", [C, CQ], F32, kind="ExternalInput").ap()
    wv_d = nc.dram_tensor("wv", [C, C], F32, kind="ExternalInput").ap()
    wt_d = nc.dram_tensor("wt", [C, C], F32, kind="ExternalInput").ap()
    vbr_d = nc.dram_tensor("vbr", [1, C], F32, kind="ExternalInput").ap()
    tb2_d = nc.dram_tensor("tb2", [C, 1], F32, kind="ExternalInput").ap()
    idn_d = nc.dram_tensor("idn", [128, 128], F32, kind="ExternalInput").ap()
    id16_d = nc.dram_tensor("id16", [128, 128], BF16, kind="ExternalInput").ap()
    on1_d = nc.dram_tensor("on1", [1, 128], F32, kind="ExternalInput").ap()
    out_d = nc.dram_tensor("out", [BPC, C, NQ], F32, kind="ExternalOutput").ap()

    with tile.TileContext(nc) as tc, ExitStack() as ctx:
        wp = ctx.enter_context(tc.tile_pool(name="w", bufs=1))
        xq_p = ctx.enter_context(tc.tile_pool(name="xq", bufs=2))
        xk_p = ctx.enter_context(tc.tile_pool(name="xk", bufs=2))
        proj_p = ctx.enter_context(tc.tile_pool(name="proj", bufs=2))
        xvt_p = ctx.enter_context(tc.tile_pool(name="xvt", bufs=2))
        kpb_p = ctx.enter_context(tc.tile_pool(name="kpb", bufs=2))
        att_p = ctx.enter_context(tc.tile_pool(name="att", bufs=1))
        attT_p = ctx.enter_context(tc.tile_pool(name="attT", bufs=1))
        u_p = ctx.enter_context(tc.tile_pool(name="u", bufs=1))
        out_p = ctx.enter_context(tc.tile_pool(name="outp", bufs=2))
        st_p = ctx.enter_context(tc.tile_pool(name="st", bufs=2))
        ps2 = ctx.enter_context(tc.tile_pool(name="ps2", bufs=2, space="PSUM"))
        ps1 = ctx.enter_context(tc.tile_pool(name="ps1", bufs=4, space="PSUM"))

        # ---- replicated weights (loaded once) ----
        wq = wp.tile([128, 4, CQ], F32)    # qk_w^T as [c_part, c_chunk, d]
        wv = wp.tile([128, 4, C], F32)     # v_w^T as [c_part, c_chunk, c_out]
        wt = wp.tile([128, 4, C], F32)     # folded trans_w^T
        for j in range(4):
            nc.sync.dma_start(out=_r(wq[:, j, :]), in_=_r(wq_d[j * 128:(j + 1) * 128, :]))
            nc.scalar.dma_start(out=_r(wv[:, j, :]), in_=_r(wv_d[j * 128:(j + 1) * 128, :]))
            nc.sync.dma_start(out=_r(wt[:, j, :]), in_=_r(wt_d[j * 128:(j + 1) * 128, :]))
        vbr = wp.tile([1, C], F32)
        nc.sync.dma_start(out=_r(vbr), in_=_r(vbr_d[:, :]))
        tb2 = wp.tile([128, 4], F32)
        for j in range(4):
            nc.sync.dma_start(out=tb2[:, j:j + 1], in_=tb2_d[j * 128:(j + 1) * 128, :])
        if ATT_BF16:
            idn16 = wp.tile([128, 128], BF16)
            nc.sync.dma_start(out=idn16, in_=id16_d[:, :])
        else:
            idn = wp.tile([128, 128], F32)
            nc.sync.dma_start(out=idn, in_=idn_d[:, :])
        on1 = wp.tile([1, 128], F32)
        nc.sync.dma_start(out=_r(on1), in_=_r(on1_d[:, :]))

        S = {}  # per-batch tile state

        def phase_a(b):
            s = S[b] = {}
            xq = s["xq"] = xq_p.tile([128, 4, NQ], F32, name=f"xq{b}", tag="xq")
            xk = s["xk"] = xk_p.tile([128, 4, NK], F32, name=f"xk{b}", tag="xk")
            for j in range(4):
                nc.sync.dma_start(out=_r(xq[:, j, :]),
                                  in_=_r(x_d[b, j * 128:(j + 1) * 128, 0:1024]))
                nc.scalar.dma_start(out=_r(xk[:, j, :]),
                                    in_=_r(x_d[b, j * 128:(j + 1) * 128, 1024:2048]))
            kp_sb = st_p.tile([1, NK], F32, name=f"kp{b}", tag="kp")
            nc.sync.dma_start(out=_r(kp_sb), in_=_r(kp_d[b, :, :]))

            # x_qT[d, n] = sum_c qk_w[d, c] * xq[c, n]
            xqT = s["xqT"] = proj_p.tile([128, NQ], F32, name=f"xqT{b}", tag="xqT")
            ps = ps2.tile([128, 1024], F32, name=f"psq{b}", tag="ps2")
            for h in range(2):
                for j in range(4):
                    nc.tensor.matmul(
                        out=ps[:, h * 512:(h + 1) * 512],
                        lhsT=_r(wq[:, j, :]),
                        rhs=_r(xq[:, j, h * 512:(h + 1) * 512]),
                        start=(j == 0), stop=(j == 3),
                    )
            nc.vector.tensor_copy(out=_r(xqT), in_=ps)

            # x_k[d, m] = sum_c qk_w[d, c] * xk[c, m]
            xks = s["xks"] = proj_p.tile([128, NK], F32, name=f"xks{b}", tag="xks")
            ps = ps2.tile([128, 1024], F32, name=f"psk{b}", tag="ps2")
            for h in range(2):
                for j in range(4):
                    nc.tensor.matmul(
                        out=ps[:, h * 512:(h + 1) * 512],
                        lhsT=_r(wq[:, j, :]),
                        rhs=_r(xk[:, j, h * 512:(h + 1) * 512]),
                        start=(j == 0), stop=(j == 3),
                    )
            nc.vector.tensor_copy(out=_r(xks), in_=ps)

            # x_v^T[m, c_out] = sum_c xk[c, m] * v_w[c_out, c] + v_b[c_out]
            xvT = s["xvT"] = xvt_p.tile([128, 8, 512], ATT_DT, name=f"xvT{b}", tag="xvT")
            for mc in range(8):
                psv = ps1.tile([128, 512], F32, name=f"psv{b}_{mc}", tag="ps1")
                for j in range(4):
                    nc.tensor.matmul(
                        out=psv,
                        lhsT=_r(xk[:, j, mc * 128:(mc + 1) * 128]),
                        rhs=_r(wv[:, j, :]),
                        start=(j == 0), stop=False,
                    )
                nc.tensor.matmul(  # + v_b as rank-1 (ones ⊗ v_b)
                    out=psv, lhsT=_r(on1), rhs=_r(vbr), start=False, stop=True,
                )
                dst = xvT[:, mc, :]
                nc.vector.tensor_copy(out=dst if ATT_BF16 else _r(dst), in_=psv)

            # broadcast k_proj to all 128 partitions via rank-1 matmul
            kpb = s["kpb"] = kpb_p.tile([128, NK], F32, name=f"kpb{b}", tag="kpb")
            ps = ps2.tile([128, 1024], F32, name=f"psb{b}", tag="ps2")
            for h in range(2):
                nc.tensor.matmul(
                    out=ps[:, h * 512:(h + 1) * 512],
                    lhsT=_r(on1),
                    rhs=_r(kp_sb[:, h * 512:(h + 1) * 512]),
                    start=True, stop=True,
                )
            nc.vector.tensor_copy(out=kpb, in_=ps)

        def phase_b(b):
            s = S[b]
            att = s["att"] = att_p.tile([128, 8, NK], ATT_DT, name=f"att{b}", tag="att")
            negrmax = st_p.tile([128, 8], F32, name=f"nrm{b}", tag="nrm")
            rowsum = st_p.tile([128, 8], F32, name=f"rs{b}", tag="rs")
            rinv = st_p.tile([128, 8], F32, name=f"ri{b}", tag="ri")
            for n_ in range(8):
                pe_ = ps2.tile([128, 1024], F32, name=f"pse{b}_{n_}", tag="ps2")
                for h in range(2):
                    nc.tensor.matmul(
                        out=pe_[:, h * 512:(h + 1) * 512],
                        lhsT=_r(s["xqT"][:, n_ * 128:(n_ + 1) * 128]),
                        rhs=_r(s["xks"][:, h * 512:(h + 1) * 512]),
                        start=True, stop=True,
                    )
                # pe_ <- e - k_proj (in place), then -rowmax via negated reduce
                nc.vector.tensor_sub(out=pe_, in0=pe_, in1=s["kpb"])
                nc.vector.tensor_reduce(
                    out=negrmax[:, n_:n_ + 1], in_=pe_, op=ALU.max,
                    axis=mybir.AxisListType.X, negate=True,
                )
                # att = exp((e-k) - rmax); rowsum via ACT accumulator
                nc.scalar.activation(
                    out=att[:, n_, :], in_=pe_, func=AF.Exp,
                    bias=negrmax[:, n_:n_ + 1], scale=1.0,
                    accum_out=rowsum[:, n_:n_ + 1],
                )
                nc.vector.reciprocal(out=rinv[:, n_:n_ + 1], in_=rowsum[:, n_:n_ + 1])
                # normalize rows on GPSIMD (otherwise idle)
                nc.gpsimd.tensor_scalar_mul(out=att[:, n_, :], in0=att[:, n_, :],
                                            scalar1=rinv[:, n_:n_ + 1])

        def phase_c(b):
            s = S[b]
            attT = s["attT"] = attT_p.tile([128, 8, NQ], ATT_DT, name=f"attT{b}", tag="attT")
            att = s["att"]
            if ATT_BF16:
                for mc in range(8):
                    pt = ps1.tile([128, 1024], BF16, name=f"pst{b}_{mc}", tag="ps1")
                    for n_ in range(8):
                        nc.tensor.matmul(
                            out=pt[:, n_ * 128:(n_ + 1) * 128],
                            lhsT=att[:, n_, mc * 128:(mc + 1) * 128],
                            rhs=idn16,
                            is_transpose=True,
                            start=(n_ == 0), stop=(n_ == 7),
                        )
                    nc.vector.tensor_copy(out=attT[:, mc, :], in_=pt)
            else:
                for mc in range(8):
                    for g in range(2):
                        pt = ps1.tile([128, 512], F32, name=f"pst{b}_{mc}_{g}", tag="ps1")
                        for k in range(4):
                            n_ = g * 4 + k
                            nc.tensor.matmul(
                                out=pt[:, k * 128:(k + 1) * 128],
                                lhsT=att[:, n_, mc * 128:(mc + 1) * 128],
                                rhs=idn,
                                is_transpose=True,
                                start=(k == 0), stop=(k == 3),
                            )
                        nc.vector.tensor_copy(
                            out=_r(attT[:, mc, g * 512:(g + 1) * 512]), in_=pt)
            colsum = st_p.tile([128, 8], F32, name=f"cs{b}", tag="cs")
            dinv = st_p.tile([128, 8], F32, name=f"di{b}", tag="di")
            nc.vector.tensor_reduce(out=colsum, in_=attT, op=ALU.add,
                                    axis=mybir.AxisListType.X)
            nc.vector.tensor_scalar_add(out=colsum, in0=colsum, scalar1=1e-9)
            nc.vector.reciprocal(out=dinv, in_=colsum)
            xvT = s["xvT"]
            for mc in range(8):
                dst = xvT[:, mc, :]
                nc.scalar.mul(out=dst if ATT_BF16 else _r(dst), in_=xvT[:, mc, :],
                              mul=dinv[:, mc:mc + 1])

        def phase_d(b):
            s = S[b]
            xq, xvT, attT = s["xq"], s["xvT"], s["attT"]
            u = u_p.tile([128, 4, NQ], F32, name=f"u{b}", tag="u")
            for cc in range(4):
                pr = ps2.tile([128, 1024], F32, name=f"psr{b}_{cc}", tag="ps2")
                for h in range(2):
                    for mc in range(8):
                        lhs = xvT[:, mc, cc * 128:(cc + 1) * 128]
                        rhs = attT[:, mc, h * 512:(h + 1) * 512]
                        nc.tensor.matmul(
                            out=pr[:, h * 512:(h + 1) * 512],
                            lhsT=lhs if ATT_BF16 else _r(lhs),
                            rhs=rhs if ATT_BF16 else _r(rhs),
                            start=(mc == 0), stop=(mc == 7),
                        )
                nc.vector.tensor_sub(out=_r(u[:, cc, :]), in0=xq[:, cc, :], in1=pr)

            for cc in range(4):
                pt2 = ps2.tile([128, 1024], F32, name=f"pso{b}_{cc}", tag="ps2")
                for h in range(2):
                    for j in range(4):
                        nc.tensor.matmul(
                            out=pt2[:, h * 512:(h + 1) * 512],
                            lhsT=_r(wt[:, j, cc * 128:(cc + 1) * 128]),
                            rhs=_r(u[:, j, h * 512:(h + 1) * 512]),
                            start=(j == 0), stop=(j == 3),
                        )
                ot = out_p.tile([128, NQ], F32, name=f"ot{b}_{cc}", tag="ot")
                nc.scalar.activation(out=ot, in_=pt2, func=AF.Relu,
                                     bias=tb2[:, cc:cc + 1], scale=1.0)
                nc.gpsimd.tensor_add(out=ot, in0=ot, in1=xq[:, cc, :])
                nc.scalar.dma_start(out=out_d[b, cc * 128:(cc + 1) * 128, :], in_=ot)

        # interleaved emission: batch1's softmax overlaps batch0's heavy tail
        phase_a(0)
        phase_a(1)
        phase_b(0)
        phase_c(0)
        phase_b(1)
        phase_d(0)
        phase_c(1)
        phase_d(1)

    nc.compile()
    return nc


def _host_prep(inputs):
    x = np.asarray(inputs["x"], np.float32)
    pos = np.asarray(inputs["pos"], np.float32)
    qk_w = np.asarray(inputs["qk_w"], np.float32)
    v_w = np.asarray(inputs["v_w"], np.float32)
    v_b = np.asarray(inputs["v_b"], np.float32)
    trans_w = np.asarray(inputs["trans_w"], np.float32)
    trans_b = np.asarray(inputs["trans_b"], np.float32)
    bn_gamma = np.asarray(inputs["bn_gamma"], np.float32)
    bn_beta = np.asarray(inputs["bn_beta"], np.float32)
    bn_mean = np.asarray(inputs["bn_mean"], np.float32)
    bn_var = np.asarray(inputs["bn_var"], np.float32)
    pos_w = np.asarray(inputs["pos_w"], np.float32)

    a = bn_gamma / np.sqrt(bn_var + 1e-5)
    wt2 = a[:, None] * trans_w
    tb2 = a * trans_b + bn_beta - a * bn_mean
    # per-key positional projection; q_proj/pos_b cancel in the key softmax
    kp = np.einsum("bpm,p->bm", pos[:, :, NQ:], pos_w).astype(np.float32)

    eye = np.eye(128, dtype=np.float32)
    common = {
        "wq": np.ascontiguousarray(qk_w.T),
        "wv": np.ascontiguousarray(v_w.T),
        "wt": np.ascontiguousarray(wt2.T),
        "vbr": np.ascontiguousarray(v_b[None, :]),
        "tb2": np.ascontiguousarray(tb2[:, None]),
        "idn": eye,
        "id16": eye.astype(ml_dtypes.bfloat16),
        "on1": np.ones((1, 128), np.float32),
    }
    in_maps = []
    for i in range(NCORES):
        m = dict(common)
        m["x"] = np.ascontiguousarray(x[BPC * i:BPC * (i + 1)])
        m["kp"] = np.ascontiguousarray(kp[BPC * i:BPC * (i + 1)][:, None, :])
        in_maps.append(m)
    return in_maps


_PROGRAM = None


def kernel(**inputs):
    global _PROGRAM
    in_maps = _host_prep(inputs)
    if _PROGRAM is None:
        _PROGRAM = build_program()
    res = run_bass_kernel_spmd(_PROGRAM, in_maps, list(range(NCORES)))
    out = np.concatenate([r["out"] for r in res.results], axis=0)
    return np.ascontiguousarray(out, dtype=np.float32)
